# revision 5
# baseline (speedup 1.0000x reference)
"""48-bit barrel right shifter with sticky bit — TRN2 Bass kernel.

Algorithm (exact fp32/int32 arithmetic, validated against the bit-level
reference):
  1. Pack each word's 48 {0,1} bits into two exact 24-bit integers (hi, lo)
     with a Horner doubling scan along the free dim (tensor_tensor_scan:
     state = pattern*state + bit, pattern = 0 at each 24-bit group start,
     else 2).
  2. Compute p = 2^-s from the 6 shift bits as a product of exact per-layer
     factors (the 2^-32 layer is the square of a 1-2^-16 factor since
     1-2^-32 is not representable in fp32).
  3. Shift arithmetically: q = hi*p, r = lo*p. The shifted word's high half
     is floor(q) = q - (q mod 1); the low half is
     (frac(q)*2^24 - frac(q*2^24)) + floor(r), all exact in fp32.
     sticky = (frac(r) > 0) OR (frac(q*2^24) != 0).
     (mod is only ISA-legal on tensor_tensor, so mods use a ones tile.)
  4. Convert the two halves to int32 (exact: integer-valued) and unpack the
     output bits with one fused TensorScalarBitvec per bit:
     bit = (v >> r) & 1, written to a strided column of an int32 out tile.
  5. The out tile is cast int32->fp32 during the output DMA (SWDGE).

Sharding: trivially data-parallel over the batch dim across 8 cores (SPMD,
same program, per-core input slices).
"""

import numpy as np

import concourse.bass as bass
import concourse.bacc as bacc
import concourse.mybir as mybir
import concourse.tile as tile
from concourse.bass_utils import run_bass_kernel_spmd

F32 = mybir.dt.float32
I32 = mybir.dt.int32
ALU = mybir.AluOpType

DB = 48          # data bits per word
SB = 6           # shift bits per word
P = 128          # SBUF partitions

B_TOTAL = 1048576
N_CORES = 8
N_CORE = B_TOTAL // N_CORES   # words per core


def build_nc(n_core=N_CORE, w=128, pool_offload=True, debug=False):
    """Build the SPMD per-core program.

    n_core:       words handled by each core.
    w:            words per partition per tile (tile free width = w*48).
    pool_offload: run f32 add/sub/mult tensor_tensor ops on GPSIMD (Pool).
    """
    tile_words = P * w
    assert n_core % tile_words == 0
    n_tiles = n_core // tile_words
    fw = w * DB

    nc = bacc.Bacc(
        "TRN2",
        target_bir_lowering=False,
        debug=debug,
        num_devices=N_CORES,
    )

    x_d = nc.dram_tensor("x", [n_core, DB], F32, kind="ExternalInput").ap()
    sh_d = nc.dram_tensor("sh", [n_core, SB], F32, kind="ExternalInput").ap()
    y_d = nc.dram_tensor("y", [n_core, DB], F32, kind="ExternalOutput").ap()
    st_d = nc.dram_tensor("st", [n_core, 1], F32, kind="ExternalOutput").ap()

    xv = x_d.rearrange("(t p w) b -> t p (w b)", p=P, w=w)
    shv = sh_d.rearrange("(t p w) s -> t p (w s)", p=P, w=w)
    yv = y_d.rearrange("(t p w) b -> t p (w b)", p=P, w=w)
    stv = st_d.rearrange("(t p w) u -> t p (w u)", p=P, w=w)

    c16 = float(1.0 - 2.0 ** -16)
    c8 = float(1.0 - 2.0 ** -8)
    c4 = float(1.0 - 2.0 ** -4)
    c2 = 0.75
    p24 = float(2.0 ** 24)

    with tile.TileContext(nc) as tc:
        with (
            tc.tile_pool(name="const", bufs=1) as cpool,
            tc.tile_pool(name="big", bufs=2) as big,
            tc.tile_pool(name="vbuf", bufs=1) as vbuf,
            tc.tile_pool(name="small", bufs=2) as small,
        ):
            # Scan multiplier pattern: 2 everywhere, 0 at the start of each
            # 24-bit group (bit positions 0 and 24 of each word).
            pattern = cpool.tile([P, fw], F32)
            nc.vector.memset(pattern[:], 2.0)
            pat3 = pattern[:].rearrange("p (g c) -> p g c", c=24)
            nc.vector.memset(pat3[:, :, 0], 0.0)

            # engine used for f32 tensor-tensor add/sub/mult
            tte = nc.gpsimd if pool_offload else nc.vector

            for t in range(n_tiles):
                xt = big.tile([P, fw], F32, tag="xt")
                nc.sync.dma_start(out=xt[:], in_=xv[t])
                sht = small.tile([P, w * SB], F32, tag="sht")
                nc.sync.dma_start(out=sht[:], in_=shv[t])

                # ---- pack: Horner doubling scan ----
                vt = vbuf.tile([P, fw], F32, tag="vt")
                nc.vector.tensor_tensor_scan(
                    out=vt[:], data0=pattern[:], data1=xt[:],
                    initial=0.0, op0=ALU.mult, op1=ALU.add,
                )
                v3 = vt[:].rearrange("p (w b) -> p w b", b=DB)
                hi = v3[:, :, 23]   # [P, w] stride 48
                lo = v3[:, :, 47]

                # ---- p = 2^-s ----
                s3 = sht[:].rearrange("p (w s) -> p w s", s=SB)

                def ts(out, in0, s1, s2, o0, o1):
                    nc.vector.tensor_scalar(
                        out=out, in0=in0, scalar1=s1, scalar2=s2, op0=o0, op1=o1
                    )

                u0 = small.tile([P, w], F32, tag="u0")
                ts(u0[:], s3[:, :, 0], c16, 1.0, ALU.mult, ALU.subtract)
                m32 = small.tile([P, w], F32, tag="m32")
                tte.tensor_mul(out=m32[:], in0=u0[:], in1=u0[:])
                m16 = small.tile([P, w], F32, tag="m16")
                ts(m16[:], s3[:, :, 1], c16, 1.0, ALU.mult, ALU.subtract)
                m8 = small.tile([P, w], F32, tag="m8")
                ts(m8[:], s3[:, :, 2], c8, 1.0, ALU.mult, ALU.subtract)
                m4 = small.tile([P, w], F32, tag="m4")
                ts(m4[:], s3[:, :, 3], c4, 1.0, ALU.mult, ALU.subtract)
                m2 = small.tile([P, w], F32, tag="m2")
                ts(m2[:], s3[:, :, 4], c2, 1.0, ALU.mult, ALU.subtract)
                m1 = small.tile([P, w], F32, tag="m1")
                ts(m1[:], s3[:, :, 5], -0.5, 1.0, ALU.mult, ALU.add)

                pa = small.tile([P, w], F32, tag="pa")
                tte.tensor_mul(out=pa[:], in0=m32[:], in1=m16[:])
                pb = small.tile([P, w], F32, tag="pb")
                tte.tensor_mul(out=pb[:], in0=m8[:], in1=m4[:])
                pc = small.tile([P, w], F32, tag="pc")
                tte.tensor_mul(out=pc[:], in0=m2[:], in1=m1[:])
                pd = small.tile([P, w], F32, tag="pd")
                tte.tensor_mul(out=pd[:], in0=pa[:], in1=pb[:])
                pw = small.tile([P, w], F32, tag="pw")
                tte.tensor_mul(out=pw[:], in0=pc[:], in1=pd[:])

                # ---- value ops ----
                # floor(x) for 0 <= x < 2^24 without mod: round-trip through
                # int32 (n = f32(i32(x)), any rounding mode), then correct:
                # floor = n - (n > x).
                def floorv(x, tag):
                    xi = small.tile([P, w], I32, tag=tag + "_i")
                    nc.vector.tensor_copy(out=xi[:], in_=x[:])
                    xn = small.tile([P, w], F32, tag=tag + "_n")
                    nc.vector.tensor_copy(out=xn[:], in_=xi[:])
                    xd = small.tile([P, w], F32, tag=tag + "_d")
                    nc.vector.tensor_tensor(out=xd[:], in0=xn[:], in1=x[:], op=ALU.is_gt)
                    xf = small.tile([P, w], F32, tag=tag + "_f")
                    tte.tensor_sub(out=xf[:], in0=xn[:], in1=xd[:])
                    return xf

                q = small.tile([P, w], F32, tag="q")
                tte.tensor_mul(out=q[:], in0=hi, in1=pw[:])
                r = small.tile([P, w], F32, tag="r")
                tte.tensor_mul(out=r[:], in0=lo, in1=pw[:])

                b1 = floorv(q, "q")                      # floor(q) = hi'
                fq = small.tile([P, w], F32, tag="fq")
                tte.tensor_sub(out=fq[:], in0=q[:], in1=b1[:])
                g = small.tile([P, w], F32, tag="g")     # frac(q)*2^24 < 2^24
                nc.vector.tensor_scalar_mul(out=g[:], in0=fq[:], scalar1=p24)
                t1 = floorv(g, "g")                      # = fq*2^24 - frac(q*2^24)
                f2 = small.tile([P, w], F32, tag="f2")
                tte.tensor_sub(out=f2[:], in0=g[:], in1=t1[:])   # frac(q*2^24)
                a2 = floorv(r, "r")                      # floor(r)
                fr = small.tile([P, w], F32, tag="fr")
                tte.tensor_sub(out=fr[:], in0=r[:], in1=a2[:])
                lo2 = small.tile([P, w], F32, tag="lo2")
                tte.tensor_add(out=lo2[:], in0=t1[:], in1=a2[:])

                qi = small.tile([P, w], I32, tag="qi")
                nc.vector.tensor_copy(out=qi[:], in_=b1[:])
                li = small.tile([P, w], I32, tag="li")
                nc.vector.tensor_copy(out=li[:], in_=lo2[:])

                stt = small.tile([P, w], F32, tag="stt")
                nc.vector.scalar_tensor_tensor(
                    out=stt[:], in0=fr[:], scalar=0.0, in1=f2[:],
                    op0=ALU.is_gt, op1=ALU.logical_or,
                )
                nc.sync.dma_start(out=stv[t], in_=stt[:])

                # ---- unpack 48 bits: (v >> r) & 1 ----
                out_t = big.tile([P, fw], I32, tag="out")
                o3 = out_t[:].rearrange("p (w b) -> p w b", b=DB)
                for j in range(DB):
                    rb = (23 - j) if j < 24 else (47 - j)
                    src = qi if j < 24 else li
                    nc.vector.tensor_scalar(
                        out=o3[:, :, j], in0=src[:],
                        scalar1=int(rb), scalar2=int(1),
                        op0=ALU.logical_shift_right, op1=ALU.bitwise_and,
                    )
                # int32 -> f32 cast happens in the DMA (SWDGE)
                nc.gpsimd.dma_start(out=yv[t], in_=out_t[:])

    nc.compile()
    return nc


_CACHE = {}


def _get_nc():
    if "nc" not in _CACHE:
        _CACHE["nc"] = build_nc()
    return _CACHE["nc"]


def kernel(X, shift):
    X = np.ascontiguousarray(np.asarray(X), dtype=np.float32)
    shift = np.ascontiguousarray(np.asarray(shift), dtype=np.float32)
    assert X.shape == (B_TOTAL, DB) and shift.shape == (B_TOTAL, SB)
    nc = _get_nc()
    in_maps = [
        {"x": X[i * N_CORE:(i + 1) * N_CORE], "sh": shift[i * N_CORE:(i + 1) * N_CORE]}
        for i in range(N_CORES)
    ]
    res = run_bass_kernel_spmd(nc, in_maps, list(range(N_CORES))).results
    y = np.concatenate([res[i]["y"] for i in range(N_CORES)], axis=0)
    st = np.concatenate([res[i]["st"] for i in range(N_CORES)], axis=0)
    return y, st
